# revision 15
# baseline (speedup 1.0000x reference)
"""Trainium2 Bass kernel for nn_CrispToFuzzyConv (hypergraph message passing).

v2: segment sums computed on the PE as one-hot matmuls (no DMA
scatter-adds, no DRAM accumulators, no zeroing):

  Stage A (edges sharded, 2 regions x 25 blocks of 128 edges/core):
    per (block, X-chunk): dma_gather 256 token slots of X[vertex]
    (tokens grouped by edge block; -1 padding costs no packets);
    onehot[p,j,c] = (c == local_edge_id[p,j]) built by one DVE
    tensor_tensor is_equal with broadcast APs;
    Xe_block = sum_j onehot_j^T @ dat_j accumulated in PSUM (f32r),
    stored to xe_sum. AllGather per region -> xe_tbl[r] [25600,128].
  Stage C (nodes sharded, 98 blocks of 128 nodes/core), two passes so
    the region-1 AllGather hides behind region-0 work:
    pass 1: gather 256 slots of Xe[edges] from xe_tbl[0], 2 matmuls
      dat_j^T @ onehot_j -> partial Xv2^T tile, parked in SBUF.
    pass 2: same for region 1 into PSUM, then the dense head:
      h2T = psum + partial (Xv2^T), h1T = transpose(deg * X tile),
      a*T = |.|, 6 f32r matmuls with [256,128] weights split in two,
      biases folded (bias_l = b_b - b_a with w_a negated), out3 write.

Known hardware constraints baked in:
  - gather indices are int16 -> X gathered in 4 chunks of 25000 rows;
    xe_tbl capped at 25600 rows; <= 1024 indices per call
  - gather layout: token t -> partition t%128, column-block t//128
  - trailing -1 indices are skipped (free padding); pad slots read
    stale SBUF, so dat pool buffers are memset once (0 * garbage
    would still be NaN if SBUF powers up with NaN bit patterns)
  - collective in/out tensors must be Internal, addr_space Local
"""

import os
import numpy as np

# ---------------------------------------------------------------- constants
N = 100000
E = 50000
NNZ = 300000
F = 128
NC = 8

EDGE_SH = E // NC            # 6250
NODE_SH = N // NC            # 12500
REG = EDGE_SH // 2           # 3125 edges per region
BLK_A = 25                   # 128-edge blocks per (core, region)
ROWS_REG = BLK_A * 128       # 3200 padded rows per (core, region)
XE_TBL = NC * ROWS_REG       # 25600 rows per region table (int16-safe)
CH = 4                       # X chunks (int16 gather limit)
CHROWS = N // CH             # 25000
KA = 256                     # slots per stage-A (block, chunk) gather
KC = 256                     # slots per stage-C (block, region) gather
BLK_C = 98                   # 128-node blocks per core
NODE_SH_P = BLK_C * 128      # 12544
SUB_A = 2 * CH               # 8 subtiles per A block
SUB_C = 4                    # 4 subtiles per C block
NG_A = 2 * BLK_A * CH        # 200 stage-A gather calls per core
NG_C = BLK_C * 2             # 196 stage-C gather calls per core

_STATE = {}


# ---------------------------------------------------------------- host side
def _wrap16(idx):
    """[n, K] int -> [n, 128, K//16] int16 (idx i at partition i%16, col
    i//16; replicated across the 8 groups of 16 partitions)."""
    n, K = idx.shape
    t = idx.reshape(n, K // 16, 16).transpose(0, 2, 1).astype(np.int16)
    return np.ascontiguousarray(np.tile(t, (1, 8, 1)))


def _route(vertex, edges):
    """Per-core gather idx + onehot rowid tensors, or None if any static
    capacity is exceeded (then the numpy fallback runs)."""
    le = edges % EDGE_SH
    owner_a = edges // EDGE_SH
    reg = le // REG
    loc_r = le - reg * REG
    blk_a = loc_r // 128
    row_a = (loc_r - blk_a * 128).astype(np.float32)
    chunk = vertex // CHROWS
    gidx_a = vertex - chunk * CHROWS
    owner_c = vertex // NODE_SH
    loc_c = vertex - owner_c * NODE_SH
    blk_c = loc_c // 128
    row_c = (loc_c - blk_c * 128).astype(np.float32)
    gidx_c = owner_a * ROWS_REG + loc_r

    out = []
    for m in range(NC):
        ia = np.full((NG_A, KA), -1, np.int64)
        na = np.zeros(NG_A, np.int64)
        rowa = np.full((NG_A // CH * SUB_A, 128), -1.0, np.float32)
        sel = np.nonzero(owner_a == m)[0]
        key = (reg[sel] * BLK_A + blk_a[sel]) * CH + chunk[sel]
        order = np.argsort(key, kind="stable")
        sel, ks = sel[order], key[order]
        starts = np.searchsorted(ks, np.arange(NG_A + 1))
        for g in range(NG_A):
            s = sel[starts[g]:starts[g + 1]]
            n = len(s)
            if n > KA:
                return None
            ia[g, :n] = gidx_a[s]
            na[g] = n
            rb, c = g // CH, g % CH
            slot = np.arange(n)
            rowa[rb * SUB_A + 2 * c + slot // 128, slot % 128] = row_a[s]
        ic = np.full((NG_C, KC), -1, np.int64)
        ncnt = np.zeros(NG_C, np.int64)
        rowc = np.full((BLK_C * SUB_C, 128), -1.0, np.float32)
        sel = np.nonzero(owner_c == m)[0]
        key = blk_c[sel] * 2 + reg[sel]
        order = np.argsort(key, kind="stable")
        sel, ks = sel[order], key[order]
        starts = np.searchsorted(ks, np.arange(NG_C + 1))
        for g in range(NG_C):
            s = sel[starts[g]:starts[g + 1]]
            n = len(s)
            if n > KC:
                return None
            ic[g, :n] = gidx_c[s]
            ncnt[g] = n
            b, r = g // 2, g % 2
            slot = np.arange(n)
            rowc[b * SUB_C + 2 * r + slot // 128, slot % 128] = row_c[s]
        out.append({
            "ia": ia, "na": na, "rowa": rowa,
            "ic": ic, "nc": ncnt, "rowc": rowc,
        })
    # equalize per-call counts across cores: num_idxs_reg is baked into
    # the (single, SPMD) program, so every core must issue the same
    # number of descriptors per call. Pad shorter cores with idx 0
    # (rowid stays -1 -> zero onehot column -> no contribution).
    cnt_a = np.maximum(np.max([o["na"] for o in out], axis=0), 16)
    cnt_c = np.maximum(np.max([o["nc"] for o in out], axis=0), 16)
    for o in out:
        for g in range(NG_A):
            o["ia"][g, o["na"][g]:cnt_a[g]] = 0
        for g in range(NG_C):
            o["ic"][g, o["nc"][g]:cnt_c[g]] = 0
        o["ia"] = _wrap16(o["ia"])
        o["ic"] = _wrap16(o["ic"])
        o["rowa"] = np.ascontiguousarray(o["rowa"].T)
        o["rowc"] = np.ascontiguousarray(o["rowc"].T)
    return out, cnt_a, cnt_c


def _numpy_fallback(X, vertex, edges, w_b, w_a, w_c, b_b, b_a, b_c):
    Xe = np.zeros((E, F), np.float32)
    np.add.at(Xe, edges, X[vertex])
    Xv2 = np.zeros((N, F), np.float32)
    np.add.at(Xv2, vertex, Xe[edges])
    deg = np.bincount(vertex, minlength=N).astype(np.float32)[:, None]
    Xv = np.concatenate([deg * X, Xv2], axis=1)
    center = Xv @ w_b + b_b
    aXv = np.abs(Xv)
    return (center.astype(np.float32),
            (center - (aXv @ w_a + b_a)).astype(np.float32),
            (center + (aXv @ w_c + b_c)).astype(np.float32))


# ------------------------------------------------------------- bass program
def _build_program(cnt_a, cnt_c):
    from concourse import bacc, tile
    import concourse.mybir as mybir

    f32 = mybir.dt.float32
    f32r = mybir.dt.float32r
    i16 = mybir.dt.int16

    nc = bacc.Bacc(None, target_bir_lowering=False, debug=False,
                   num_devices=NC, num_swdge_queues=4)

    xfull = nc.dram_tensor("xfull", [N, F], f32r, kind="ExternalInput")
    xshard = nc.dram_tensor("xshard", [NODE_SH_P, F], f32, kind="ExternalInput")
    ia = nc.dram_tensor("ia", [NG_A, 128, KA // 16], i16, kind="ExternalInput")
    ic = nc.dram_tensor("ic", [NG_C, 128, KC // 16], i16, kind="ExternalInput")
    rowa_d = nc.dram_tensor("rowa", [128, NG_A // CH * SUB_A], f32,
                            kind="ExternalInput")
    rowc_d = nc.dram_tensor("rowc", [128, BLK_C * SUB_C], f32,
                            kind="ExternalInput")
    deg = nc.dram_tensor("deg", [128, BLK_C], f32, kind="ExternalInput")
    wts_d = {nm: nc.dram_tensor(nm, [F, F], f32r, kind="ExternalInput")
             for nm in ("wb1", "wb2", "wa1n", "wa2n", "wc1", "wc2")}
    bias_d = {nm: nc.dram_tensor(nm, [1, F], f32, kind="ExternalInput")
              for nm in ("bias_c", "bias_l", "bias_r")}
    out3 = nc.dram_tensor("out3", [NODE_SH_P, 3 * F], f32,
                          kind="ExternalOutput")

    xe_sum = nc.dram_tensor("xe_sum", [2 * ROWS_REG, F], f32r)
    xe_tbl = [nc.dram_tensor(f"xe_tbl{r}", [XE_TBL, F], f32r)
              for r in range(2)]

    eye_d = nc.inline_tensor(np.eye(128, dtype=np.float32), name="eye128")
    ramp_np = np.broadcast_to(
        np.arange(128, dtype=np.float32),
        (128, SUB_A, 128)).copy()
    ramp_d = nc.inline_tensor(ramp_np, name="ramp8")

    ISEQ = mybir.AluOpType.is_equal
    Abs = mybir.ActivationFunctionType.Abs
    Copy = mybir.ActivationFunctionType.Copy

    with tile.TileContext(nc) as tc:
        with (
            tc.tile_pool(name="cpool", bufs=1) as cpool,
            tc.tile_pool(name="ppool", bufs=1) as ppool,
            tc.tile_pool(name="ipool", bufs=8) as ipool,
            tc.tile_pool(name="dpa", bufs=3) as dpa,
            tc.tile_pool(name="oha", bufs=2) as oha,
            tc.tile_pool(name="dpc", bufs=3) as dpc,
            tc.tile_pool(name="ohc", bufs=2) as ohc,
            tc.tile_pool(name="spool", bufs=4) as spool,
            tc.tile_pool(name="opool", bufs=3) as opool,
            tc.tile_pool(name="ps_sg", bufs=3, space="PSUM") as ps_sg,
            tc.tile_pool(name="ps_tr", bufs=2, space="PSUM") as ps_tr,
            tc.tile_pool(name="ps_mm", bufs=3, space="PSUM") as ps_mm,
        ):
            # constants
            ident = cpool.tile([128, 128], f32)
            nc.sync.dma_start(ident[:], eye_d[:])
            ramp = cpool.tile([128, SUB_A, 128], f32)
            nc.sync.dma_start(ramp[:], ramp_d[:])
            rowa_s = cpool.tile([128, NG_A // CH * SUB_A], f32)
            nc.sync.dma_start(rowa_s[:], rowa_d[:])
            rowc_s = cpool.tile([128, BLK_C * SUB_C], f32)
            nc.sync.dma_start(rowc_s[:], rowc_d[:])
            deg_all = cpool.tile([128, BLK_C], f32)
            nc.sync.dma_start(deg_all[:], deg[:])
            ones = cpool.tile([1, F], f32)
            nc.vector.memset(ones[:], 1.0)
            wts = {}
            for nm, d in wts_d.items():
                wtile = cpool.tile([F, F], f32r, tag=nm)
                nc.sync.dma_start(wtile[:], d[:])
                wts[nm] = wtile
            bias_bc = {}
            bmm = ps_mm.tile([128, 3 * F], f32, tag="mm3")
            for k, (nm, d) in enumerate(bias_d.items()):
                btile = cpool.tile([1, F], f32, tag=nm)
                nc.sync.dma_start(btile[:], d[:])
                nc.tensor.matmul(bmm[:, k * F:(k + 1) * F], ones[:], btile[:],
                                 start=True, stop=True)
            for k, nm in enumerate(bias_d):
                bct = cpool.tile([128, F], f32, tag=f"bc_{nm}")
                nc.vector.tensor_copy(bct[:], bmm[:, k * F:(k + 1) * F])
                bias_bc[nm] = bct
            # pre-zero the gather data pools (pad slots are never written;
            # 0 * stale-NaN would poison PSUM)
            for _ in range(3):
                t = dpa.tile([128, SUB_A, F], f32r, tag="datA")
                nc.vector.memset(t[:].bitcast(f32), 0.0)
                t = dpc.tile([128, 2, F], f32r, tag="datC")
                nc.vector.memset(t[:].bitcast(f32), 0.0)

            def cc(r):
                lo, hi = r * ROWS_REG, (r + 1) * ROWS_REG
                nc.gpsimd.collective_compute(
                    "AllGather", mybir.AluOpType.bypass,
                    replica_groups=[list(range(NC))],
                    ins=[xe_sum[lo:hi, :].opt()],
                    outs=[xe_tbl[r].ap().opt()],
                )

            # stage A: Xe blocks via onehot matmuls
            for r in range(2):
                for b in range(BLK_A):
                    dat = dpa.tile([128, SUB_A, F], f32r, tag="datA")
                    for c in range(CH):
                        g = (r * BLK_A + b) * CH + c
                        it = ipool.tile([128, KA // 16], i16, tag="ita")
                        nc.sync.dma_start(it[:], ia[g])
                        nc.gpsimd.dma_gather(
                            dat[:, 2 * c:2 * c + 2, :],
                            xfull[c * CHROWS:(c + 1) * CHROWS, :],
                            it[:], KA, int(cnt_a[g]), F, queue_num=g % 4)
                    oh = oha.tile([128, SUB_A, 128], f32r, tag="ohA")
                    g0 = (r * BLK_A + b) * SUB_A
                    nc.vector.tensor_tensor(
                        oh[:], ramp[:],
                        rowa_s[:, g0:g0 + SUB_A].unsqueeze(2).broadcast_to(
                            (128, SUB_A, 128)),
                        ISEQ)
                    ps = ps_sg.tile([128, F], f32, tag="sg")
                    for j in range(SUB_A):
                        nc.tensor.matmul(ps[:], oh[:, j, :], dat[:, j, :],
                                         start=(j == 0), stop=(j == SUB_A - 1))
                    st = spool.tile([128, F], f32r, tag="xe_st")
                    nc.scalar.activation(st[:], ps[:], Copy)
                    row0 = r * ROWS_REG + b * 128
                    nc.scalar.dma_start(xe_sum[row0:row0 + 128, :], st[:])
                    if r == 1 and b == 1:
                        cc(0)
            cc(1)

            # stage C pass 1: region-0 partial Xv2^T into SBUF
            parts = []
            for b in range(BLK_C):
                dat = dpc.tile([128, 2, F], f32r, tag="datC")
                it = ipool.tile([128, KC // 16], i16, tag="itc")
                nc.sync.dma_start(it[:], ic[b * 2])
                nc.gpsimd.dma_gather(dat[:], xe_tbl[0][:], it[:],
                                     KC, int(cnt_c[b * 2]), F,
                                     queue_num=(b * 2) % 4)
                oh = ohc.tile([128, 2, 128], f32r, tag="ohC")
                nc.vector.tensor_tensor(
                    oh[:], ramp[:, 0:2, :],
                    rowc_s[:, b * SUB_C:b * SUB_C + 2].unsqueeze(2)
                    .broadcast_to((128, 2, 128)),
                    ISEQ)
                ps = ps_sg.tile([128, 128], f32, tag="sg")
                for j in range(2):
                    nc.tensor.matmul(ps[:], dat[:, j, :], oh[:, j, :],
                                     start=(j == 0), stop=(j == 1))
                part = ppool.tile([128, 128], f32, tag=f"part{b}")
                nc.vector.tensor_copy(part[:], ps[:])
                parts.append(part)

            # stage C pass 2 + dense head
            for b in range(BLK_C):
                part = parts[b]
                dat = dpc.tile([128, 2, F], f32r, tag="datC")
                it = ipool.tile([128, KC // 16], i16, tag="itc")
                nc.sync.dma_start(it[:], ic[b * 2 + 1])
                nc.gpsimd.dma_gather(dat[:], xe_tbl[1][:], it[:],
                                     KC, int(cnt_c[b * 2 + 1]), F,
                                     queue_num=(b * 2 + 1) % 4)
                oh = ohc.tile([128, 2, 128], f32r, tag="ohC")
                nc.vector.tensor_tensor(
                    oh[:], ramp[:, 0:2, :],
                    rowc_s[:, b * SUB_C + 2:b * SUB_C + 4].unsqueeze(2)
                    .broadcast_to((128, 2, 128)),
                    ISEQ)
                ps = ps_sg.tile([128, 128], f32, tag="sg")
                for j in range(2):
                    nc.tensor.matmul(ps[:], dat[:, j, :], oh[:, j, :],
                                     start=(j == 0), stop=(j == 1))
                h2T = spool.tile([128, 128], f32r, tag="h2T")
                nc.vector.tensor_add(h2T[:], ps[:], part[:])
                a2T = spool.tile([128, 128], f32r, tag="a2T")
                nc.scalar.activation(a2T[:], h2T[:], Abs)

                r0 = b * 128
                xt = spool.tile([128, F], f32, tag="xt")
                nc.sync.dma_start(xt[:], xshard[r0:r0 + 128, :])
                h1 = spool.tile([128, F], f32, tag="h1")
                nc.vector.tensor_scalar_mul(h1[:], xt[:], deg_all[:, b:b + 1])
                h1T_ps = ps_tr.tile([128, F], f32, tag="h1T_ps")
                nc.tensor.transpose(h1T_ps[:], h1[:], ident[:])
                h1T = spool.tile([128, F], f32r, tag="h1T")
                nc.vector.tensor_copy(h1T[:], h1T_ps[:])
                a1T = spool.tile([128, F], f32r, tag="a1T")
                nc.scalar.activation(a1T[:], h1T_ps[:], Abs)

                groups = (
                    ("hsum_ps", (("h1T", "wb1"), ("h2T", "wb2"))),
                    ("lpart_ps", (("a1T", "wa1n"), ("a2T", "wa2n"))),
                    ("rpart_ps", (("a1T", "wc1"), ("a2T", "wc2"))),
                )
                lhs = {"h1T": h1T, "h2T": h2T, "a1T": a1T, "a2T": a2T}
                mm = ps_mm.tile([128, 3 * F], f32, tag="mm3")
                ps_out = {}
                for k, (psname, terms) in enumerate(groups):
                    pso = mm[:, k * F:(k + 1) * F]
                    for i, (ln, wn) in enumerate(terms):
                        nc.tensor.matmul(
                            pso, lhs[ln][:], wts[wn][:],
                            start=(i == 0), stop=(i == len(terms) - 1))
                    ps_out[psname] = pso
                ot = opool.tile([128, 3 * F], f32, tag="ot")
                nc.vector.tensor_add(ot[:, 0:F], bias_bc["bias_c"][:],
                                     ps_out["hsum_ps"])
                for k, (pname, bname) in enumerate((("lpart_ps", "bias_l"),
                                                    ("rpart_ps", "bias_r"))):
                    tmp = opool.tile([128, F], f32, tag=f"t{k}")
                    nc.vector.tensor_add(tmp[:], bias_bc[bname][:],
                                         ps_out[pname])
                    nc.vector.tensor_add(ot[:, (k + 1) * F:(k + 2) * F],
                                         tmp[:], ps_out["hsum_ps"])
                nc.sync.dma_start(out3[r0:r0 + 128, :], ot[:])

    nc.compile()
    return nc


# ------------------------------------------------------------------- driver
def kernel(X, vertex, edges, X0, n_edges, w_b, w_a, w_c, b_b, b_a, b_c):
    from concourse.bass_utils import run_bass_kernel_spmd

    X = np.ascontiguousarray(np.asarray(X, dtype=np.float32))
    vertex = np.asarray(vertex).astype(np.int64)
    edges = np.asarray(edges).astype(np.int64)
    w_b = np.asarray(w_b, dtype=np.float32)
    w_a = np.asarray(w_a, dtype=np.float32)
    w_c = np.asarray(w_c, dtype=np.float32)
    b_b = np.asarray(b_b, dtype=np.float32).reshape(1, F)
    b_a = np.asarray(b_a, dtype=np.float32).reshape(1, F)
    b_c = np.asarray(b_c, dtype=np.float32).reshape(1, F)

    r = _route(vertex, edges)
    if r is None:
        return _numpy_fallback(X, vertex, edges, w_b, w_a, w_c, b_b, b_a, b_c)
    routed, cnt_a, cnt_c = r

    key = (cnt_a.tobytes(), cnt_c.tobytes())
    if _STATE.get("key") != key:
        _STATE["nc"] = _build_program(cnt_a, cnt_c)
        _STATE["key"] = key
    nc = _STATE["nc"]

    deg_full = np.bincount(vertex, minlength=N).astype(np.float32)
    wmats = {
        "wb1": w_b[:F], "wb2": w_b[F:],
        "wa1n": -w_a[:F], "wa2n": -w_a[F:],
        "wc1": w_c[:F], "wc2": w_c[F:],
    }
    bmats = {"bias_c": b_b, "bias_l": b_b - b_a, "bias_r": b_b + b_c}

    in_maps = []
    for m in range(NC):
        xs = np.zeros((NODE_SH_P, F), np.float32)
        xs[:NODE_SH] = X[m * NODE_SH:(m + 1) * NODE_SH]
        dshard = np.zeros(NODE_SH_P, np.float32)
        dshard[:NODE_SH] = deg_full[m * NODE_SH:(m + 1) * NODE_SH]
        im = {
            "xfull": X,
            "xshard": xs,
            "ia": routed[m]["ia"], "ic": routed[m]["ic"],
            "rowa": routed[m]["rowa"], "rowc": routed[m]["rowc"],
            "deg": np.ascontiguousarray(dshard.reshape(BLK_C, 128).T),
        }
        for nm, w in wmats.items():
            im[nm] = np.ascontiguousarray(w.astype(np.float32))
        for nm, bv in bmats.items():
            im[nm] = np.ascontiguousarray(bv.astype(np.float32))
        in_maps.append(im)

    res = run_bass_kernel_spmd(nc, in_maps, list(range(NC)))
    full = np.concatenate([res.results[m]["out3"][:NODE_SH]
                           for m in range(NC)])
    full = full.reshape(N, 3, F)
    return (np.ascontiguousarray(full[:, 0]),
            np.ascontiguousarray(full[:, 1]),
            np.ascontiguousarray(full[:, 2]))


# revision 16
# speedup vs baseline: 1.1083x; 1.1083x over previous
"""Trainium2 Bass kernel for nn_CrispToFuzzyConv (hypergraph message passing).

v2: segment sums computed on the PE as one-hot matmuls (no DMA
scatter-adds, no DRAM accumulators, no zeroing):

  Stage A (edges sharded, 2 regions x 25 blocks of 128 edges/core):
    per (block, X-chunk): dma_gather 256 token slots of X[vertex]
    (tokens grouped by edge block; -1 padding costs no packets);
    onehot[p,j,c] = (c == local_edge_id[p,j]) built by one DVE
    tensor_tensor is_equal with broadcast APs;
    Xe_block = sum_j onehot_j^T @ dat_j accumulated in PSUM (f32r),
    stored to xe_sum. AllGather per region -> xe_tbl[r] [25600,128].
  Stage C (nodes sharded, 98 blocks of 128 nodes/core), two passes so
    the region-1 AllGather hides behind region-0 work:
    pass 1: gather 256 slots of Xe[edges] from xe_tbl[0], 2 matmuls
      dat_j^T @ onehot_j -> partial Xv2^T tile, parked in SBUF.
    pass 2: same for region 1 into PSUM, then the dense head:
      h2T = psum + partial (Xv2^T), h1T = transpose(deg * X tile),
      a*T = |.|, 6 f32r matmuls with [256,128] weights split in two,
      biases folded (bias_l = b_b - b_a with w_a negated), out3 write.

Known hardware constraints baked in:
  - gather indices are int16 -> X gathered in 4 chunks of 25000 rows;
    xe_tbl capped at 25600 rows; <= 1024 indices per call
  - gather layout: token t -> partition t%128, column-block t//128
  - trailing -1 indices are skipped (free padding); pad slots read
    stale SBUF, so dat pool buffers are memset once (0 * garbage
    would still be NaN if SBUF powers up with NaN bit patterns)
  - collective in/out tensors must be Internal, addr_space Local
"""

import os
import numpy as np

# ---------------------------------------------------------------- constants
N = 100000
E = 50000
NNZ = 300000
F = 128
NC = 8

EDGE_SH = E // NC            # 6250
NODE_SH = N // NC            # 12500
REG = EDGE_SH // 2           # 3125 edges per region
BLK_A = 25                   # 128-edge blocks per (core, region)
ROWS_REG = BLK_A * 128       # 3200 padded rows per (core, region)
XE_TBL = NC * ROWS_REG       # 25600 rows per region table (int16-safe)
CH = 4                       # X chunks (int16 gather limit)
CHROWS = N // CH             # 25000
KA = 256                     # slots per stage-A (block, chunk) gather
KC = 256                     # slots per stage-C (block, region) gather
BLK_C = 98                   # 128-node blocks per core
NODE_SH_P = BLK_C * 128      # 12544
SUB_A = 2 * CH               # 8 subtiles per A block
SUB_C = 4                    # 4 subtiles per C block
NG_A = 2 * BLK_A * CH        # 200 stage-A gather calls per core
NG_C = BLK_C * 2             # 196 stage-C gather calls per core

_STATE = {}


# ---------------------------------------------------------------- host side
def _wrap16(idx):
    """[n, K] int -> [n, 128, K//16] int16 (idx i at partition i%16, col
    i//16; replicated across the 8 groups of 16 partitions)."""
    n, K = idx.shape
    t = idx.reshape(n, K // 16, 16).transpose(0, 2, 1).astype(np.int16)
    return np.ascontiguousarray(np.tile(t, (1, 8, 1)))


def _route(vertex, edges):
    """Per-core gather idx + onehot rowid tensors, or None if any static
    capacity is exceeded (then the numpy fallback runs)."""
    le = edges % EDGE_SH
    owner_a = edges // EDGE_SH
    reg = le // REG
    loc_r = le - reg * REG
    blk_a = loc_r // 128
    row_a = (loc_r - blk_a * 128).astype(np.float32)
    chunk = vertex // CHROWS
    gidx_a = vertex - chunk * CHROWS
    owner_c = vertex // NODE_SH
    loc_c = vertex - owner_c * NODE_SH
    blk_c = loc_c // 128
    row_c = (loc_c - blk_c * 128).astype(np.float32)
    gidx_c = owner_a * ROWS_REG + loc_r

    out = []
    for m in range(NC):
        ia = np.full((NG_A, KA), -1, np.int64)
        na = np.zeros(NG_A, np.int64)
        rowa = np.full((NG_A // CH * SUB_A, 128), -1.0, np.float32)
        sel = np.nonzero(owner_a == m)[0]
        key = (reg[sel] * BLK_A + blk_a[sel]) * CH + chunk[sel]
        order = np.argsort(key, kind="stable")
        sel, ks = sel[order], key[order]
        starts = np.searchsorted(ks, np.arange(NG_A + 1))
        for g in range(NG_A):
            s = sel[starts[g]:starts[g + 1]]
            n = len(s)
            if n > KA:
                return None
            ia[g, :n] = gidx_a[s]
            na[g] = n
            rb, c = g // CH, g % CH
            slot = np.arange(n)
            rowa[rb * SUB_A + 2 * c + slot // 128, slot % 128] = row_a[s]
        ic = np.full((NG_C, KC), -1, np.int64)
        ncnt = np.zeros(NG_C, np.int64)
        rowc = np.full((BLK_C * SUB_C, 128), -1.0, np.float32)
        sel = np.nonzero(owner_c == m)[0]
        key = blk_c[sel] * 2 + reg[sel]
        order = np.argsort(key, kind="stable")
        sel, ks = sel[order], key[order]
        starts = np.searchsorted(ks, np.arange(NG_C + 1))
        for g in range(NG_C):
            s = sel[starts[g]:starts[g + 1]]
            n = len(s)
            if n > KC:
                return None
            ic[g, :n] = gidx_c[s]
            ncnt[g] = n
            b, r = g // 2, g % 2
            slot = np.arange(n)
            rowc[b * SUB_C + 2 * r + slot // 128, slot % 128] = row_c[s]
        out.append({
            "ia": ia, "na": na, "rowa": rowa,
            "ic": ic, "nc": ncnt, "rowc": rowc,
        })
    # equalize per-call counts across cores: num_idxs_reg is baked into
    # the (single, SPMD) program, so every core must issue the same
    # number of descriptors per call. Pad shorter cores with idx 0
    # (rowid stays -1 -> zero onehot column -> no contribution).
    cnt_a = np.maximum(np.max([o["na"] for o in out], axis=0), 16)
    cnt_c = np.maximum(np.max([o["nc"] for o in out], axis=0), 16)
    for o in out:
        for g in range(NG_A):
            o["ia"][g, o["na"][g]:cnt_a[g]] = 0
        for g in range(NG_C):
            o["ic"][g, o["nc"][g]:cnt_c[g]] = 0
        o["ia"] = _wrap16(o["ia"])
        o["ic"] = _wrap16(o["ic"])
        o["rowa"] = np.ascontiguousarray(o["rowa"].T)
        o["rowc"] = np.ascontiguousarray(o["rowc"].T)
    return out, cnt_a, cnt_c


def _numpy_fallback(X, vertex, edges, w_b, w_a, w_c, b_b, b_a, b_c):
    Xe = np.zeros((E, F), np.float32)
    np.add.at(Xe, edges, X[vertex])
    Xv2 = np.zeros((N, F), np.float32)
    np.add.at(Xv2, vertex, Xe[edges])
    deg = np.bincount(vertex, minlength=N).astype(np.float32)[:, None]
    Xv = np.concatenate([deg * X, Xv2], axis=1)
    center = Xv @ w_b + b_b
    aXv = np.abs(Xv)
    return (center.astype(np.float32),
            (center - (aXv @ w_a + b_a)).astype(np.float32),
            (center + (aXv @ w_c + b_c)).astype(np.float32))


# ------------------------------------------------------------- bass program
def _build_program(cnt_a, cnt_c):
    from concourse import bacc, tile
    import concourse.mybir as mybir

    f32 = mybir.dt.float32
    bf16 = mybir.dt.bfloat16
    i16 = mybir.dt.int16

    nc = bacc.Bacc(None, target_bir_lowering=False, debug=False,
                   num_devices=NC, num_swdge_queues=4)

    xfull = nc.dram_tensor("xfull", [N, F], bf16, kind="ExternalInput")
    xshard = nc.dram_tensor("xshard", [NODE_SH_P, F], f32, kind="ExternalInput")
    ia = nc.dram_tensor("ia", [NG_A, 128, KA // 16], i16, kind="ExternalInput")
    ic = nc.dram_tensor("ic", [NG_C, 128, KC // 16], i16, kind="ExternalInput")
    rowa_d = nc.dram_tensor("rowa", [128, NG_A // CH * SUB_A], f32,
                            kind="ExternalInput")
    rowc_d = nc.dram_tensor("rowc", [128, BLK_C * SUB_C], f32,
                            kind="ExternalInput")
    deg = nc.dram_tensor("deg", [128, BLK_C], f32, kind="ExternalInput")
    wts_d = {nm: nc.dram_tensor(nm, [F, F], bf16, kind="ExternalInput")
             for nm in ("wb1", "wb2", "wa1n", "wa2n", "wc1", "wc2")}
    bias_d = {nm: nc.dram_tensor(nm, [1, F], f32, kind="ExternalInput")
              for nm in ("bias_c", "bias_l", "bias_r")}
    out3 = nc.dram_tensor("out3", [NODE_SH_P, 3 * F], f32,
                          kind="ExternalOutput")

    xe_sum = nc.dram_tensor("xe_sum", [2 * ROWS_REG, F], bf16)
    xe_tbl = [nc.dram_tensor(f"xe_tbl{r}", [XE_TBL, F], bf16)
              for r in range(2)]

    eye_d = nc.inline_tensor(np.eye(128, dtype=np.float32), name="eye128")
    ramp_np = np.broadcast_to(
        np.arange(128, dtype=np.float32),
        (128, SUB_A, 128)).copy()
    ramp_d = nc.inline_tensor(ramp_np, name="ramp8")

    ISEQ = mybir.AluOpType.is_equal
    Abs = mybir.ActivationFunctionType.Abs
    Copy = mybir.ActivationFunctionType.Copy

    with tile.TileContext(nc) as tc:
        with (
            tc.tile_pool(name="cpool", bufs=1) as cpool,
            tc.tile_pool(name="ppool", bufs=1) as ppool,
            tc.tile_pool(name="ipool", bufs=8) as ipool,
            tc.tile_pool(name="dpa", bufs=3) as dpa,
            tc.tile_pool(name="oha", bufs=2) as oha,
            tc.tile_pool(name="dpc", bufs=3) as dpc,
            tc.tile_pool(name="ohc", bufs=2) as ohc,
            tc.tile_pool(name="spool", bufs=4) as spool,
            tc.tile_pool(name="opool", bufs=3) as opool,
            tc.tile_pool(name="ps_sg", bufs=3, space="PSUM") as ps_sg,
            tc.tile_pool(name="ps_tr", bufs=2, space="PSUM") as ps_tr,
            tc.tile_pool(name="ps_mm", bufs=3, space="PSUM") as ps_mm,
        ):
            # constants
            ident = cpool.tile([128, 128], f32)
            nc.sync.dma_start(ident[:], eye_d[:])
            ramp = cpool.tile([128, SUB_A, 128], f32)
            nc.sync.dma_start(ramp[:], ramp_d[:])
            rowa_s = cpool.tile([128, NG_A // CH * SUB_A], f32)
            nc.sync.dma_start(rowa_s[:], rowa_d[:])
            rowc_s = cpool.tile([128, BLK_C * SUB_C], f32)
            nc.sync.dma_start(rowc_s[:], rowc_d[:])
            deg_all = cpool.tile([128, BLK_C], f32)
            nc.sync.dma_start(deg_all[:], deg[:])
            ones = cpool.tile([1, F], f32)
            nc.vector.memset(ones[:], 1.0)
            wts = {}
            for nm, d in wts_d.items():
                wtile = cpool.tile([F, F], bf16, tag=nm)
                nc.sync.dma_start(wtile[:], d[:])
                wts[nm] = wtile
            bias_bc = {}
            bmm = ps_mm.tile([128, 3 * F], f32, tag="mm3")
            for k, (nm, d) in enumerate(bias_d.items()):
                btile = cpool.tile([1, F], f32, tag=nm)
                nc.sync.dma_start(btile[:], d[:])
                nc.tensor.matmul(bmm[:, k * F:(k + 1) * F], ones[:], btile[:],
                                 start=True, stop=True)
            for k, nm in enumerate(bias_d):
                bct = cpool.tile([128, F], f32, tag=f"bc_{nm}")
                nc.vector.tensor_copy(bct[:], bmm[:, k * F:(k + 1) * F])
                bias_bc[nm] = bct
            # pre-zero the gather data pools (pad slots are never written;
            # 0 * stale-NaN would poison PSUM)
            for _ in range(3):
                t = dpa.tile([128, SUB_A, F], bf16, tag="datA")
                nc.vector.memset(t[:], 0.0)
                t = dpc.tile([128, 2, F], bf16, tag="datC")
                nc.vector.memset(t[:], 0.0)

            def cc(r):
                lo, hi = r * ROWS_REG, (r + 1) * ROWS_REG
                nc.gpsimd.collective_compute(
                    "AllGather", mybir.AluOpType.bypass,
                    replica_groups=[list(range(NC))],
                    ins=[xe_sum[lo:hi, :].opt()],
                    outs=[xe_tbl[r].ap().opt()],
                )

            # stage A: Xe blocks via onehot matmuls
            for r in range(2):
                for b in range(BLK_A):
                    dat = dpa.tile([128, SUB_A, F], bf16, tag="datA")
                    for c in range(CH):
                        g = (r * BLK_A + b) * CH + c
                        it = ipool.tile([128, KA // 16], i16, tag="ita")
                        nc.sync.dma_start(it[:], ia[g])
                        nc.gpsimd.dma_gather(
                            dat[:, 2 * c:2 * c + 2, :],
                            xfull[c * CHROWS:(c + 1) * CHROWS, :],
                            it[:], KA, int(cnt_a[g]), F, queue_num=g % 4)
                    oh = oha.tile([128, SUB_A, 128], bf16, tag="ohA")
                    g0 = (r * BLK_A + b) * SUB_A
                    nc.vector.tensor_tensor(
                        oh[:], ramp[:],
                        rowa_s[:, g0:g0 + SUB_A].unsqueeze(2).broadcast_to(
                            (128, SUB_A, 128)),
                        ISEQ)
                    ps = ps_sg.tile([128, F], f32, tag="sg")
                    for j in range(SUB_A):
                        nc.tensor.matmul(ps[:], oh[:, j, :], dat[:, j, :],
                                         start=(j == 0), stop=(j == SUB_A - 1))
                    st = spool.tile([128, F], bf16, tag="xe_st")
                    nc.scalar.activation(st[:], ps[:], Copy)
                    row0 = r * ROWS_REG + b * 128
                    nc.scalar.dma_start(xe_sum[row0:row0 + 128, :], st[:])
                    if r == 1 and b == 1:
                        cc(0)
            cc(1)

            # stage C pass 1: region-0 partial Xv2^T into SBUF
            parts = []
            for b in range(BLK_C):
                dat = dpc.tile([128, 2, F], bf16, tag="datC")
                it = ipool.tile([128, KC // 16], i16, tag="itc")
                nc.sync.dma_start(it[:], ic[b * 2])
                nc.gpsimd.dma_gather(dat[:], xe_tbl[0][:], it[:],
                                     KC, int(cnt_c[b * 2]), F,
                                     queue_num=(b * 2) % 4)
                oh = ohc.tile([128, 2, 128], bf16, tag="ohC")
                nc.vector.tensor_tensor(
                    oh[:], ramp[:, 0:2, :],
                    rowc_s[:, b * SUB_C:b * SUB_C + 2].unsqueeze(2)
                    .broadcast_to((128, 2, 128)),
                    ISEQ)
                ps = ps_sg.tile([128, 128], f32, tag="sg")
                for j in range(2):
                    nc.tensor.matmul(ps[:], dat[:, j, :], oh[:, j, :],
                                     start=(j == 0), stop=(j == 1))
                part = ppool.tile([128, 128], f32, tag=f"part{b}")
                nc.vector.tensor_copy(part[:], ps[:])
                parts.append(part)

            # stage C pass 2 + dense head
            for b in range(BLK_C):
                part = parts[b]
                dat = dpc.tile([128, 2, F], bf16, tag="datC")
                it = ipool.tile([128, KC // 16], i16, tag="itc")
                nc.sync.dma_start(it[:], ic[b * 2 + 1])
                nc.gpsimd.dma_gather(dat[:], xe_tbl[1][:], it[:],
                                     KC, int(cnt_c[b * 2 + 1]), F,
                                     queue_num=(b * 2 + 1) % 4)
                oh = ohc.tile([128, 2, 128], bf16, tag="ohC")
                nc.vector.tensor_tensor(
                    oh[:], ramp[:, 0:2, :],
                    rowc_s[:, b * SUB_C + 2:b * SUB_C + 4].unsqueeze(2)
                    .broadcast_to((128, 2, 128)),
                    ISEQ)
                ps = ps_sg.tile([128, 128], f32, tag="sg")
                for j in range(2):
                    nc.tensor.matmul(ps[:], dat[:, j, :], oh[:, j, :],
                                     start=(j == 0), stop=(j == 1))
                h2T = spool.tile([128, 128], bf16, tag="h2T")
                nc.vector.tensor_add(h2T[:], ps[:], part[:])
                a2T = spool.tile([128, 128], bf16, tag="a2T")
                nc.scalar.activation(a2T[:], h2T[:], Abs)

                r0 = b * 128
                xt = spool.tile([128, F], f32, tag="xt")
                nc.sync.dma_start(xt[:], xshard[r0:r0 + 128, :])
                h1 = spool.tile([128, F], f32, tag="h1")
                nc.vector.tensor_scalar_mul(h1[:], xt[:], deg_all[:, b:b + 1])
                h1T_ps = ps_tr.tile([128, F], f32, tag="h1T_ps")
                nc.tensor.transpose(h1T_ps[:], h1[:], ident[:])
                h1T = spool.tile([128, F], bf16, tag="h1T")
                nc.vector.tensor_copy(h1T[:], h1T_ps[:])
                a1T = spool.tile([128, F], bf16, tag="a1T")
                nc.scalar.activation(a1T[:], h1T_ps[:], Abs)

                groups = (
                    ("hsum_ps", (("h1T", "wb1"), ("h2T", "wb2"))),
                    ("lpart_ps", (("a1T", "wa1n"), ("a2T", "wa2n"))),
                    ("rpart_ps", (("a1T", "wc1"), ("a2T", "wc2"))),
                )
                lhs = {"h1T": h1T, "h2T": h2T, "a1T": a1T, "a2T": a2T}
                mm = ps_mm.tile([128, 3 * F], f32, tag="mm3")
                ps_out = {}
                for k, (psname, terms) in enumerate(groups):
                    pso = mm[:, k * F:(k + 1) * F]
                    for i, (ln, wn) in enumerate(terms):
                        nc.tensor.matmul(
                            pso, lhs[ln][:], wts[wn][:],
                            start=(i == 0), stop=(i == len(terms) - 1))
                    ps_out[psname] = pso
                ot = opool.tile([128, 3 * F], f32, tag="ot")
                nc.vector.tensor_add(ot[:, 0:F], bias_bc["bias_c"][:],
                                     ps_out["hsum_ps"])
                for k, (pname, bname) in enumerate((("lpart_ps", "bias_l"),
                                                    ("rpart_ps", "bias_r"))):
                    tmp = opool.tile([128, F], f32, tag=f"t{k}")
                    nc.vector.tensor_add(tmp[:], bias_bc[bname][:],
                                         ps_out[pname])
                    nc.vector.tensor_add(ot[:, (k + 1) * F:(k + 2) * F],
                                         tmp[:], ps_out["hsum_ps"])
                nc.sync.dma_start(out3[r0:r0 + 128, :], ot[:])

    nc.compile()
    return nc


# ------------------------------------------------------------------- driver
def kernel(X, vertex, edges, X0, n_edges, w_b, w_a, w_c, b_b, b_a, b_c):
    from concourse.bass_utils import run_bass_kernel_spmd

    import ml_dtypes
    X = np.ascontiguousarray(np.asarray(X, dtype=np.float32))
    Xb = np.ascontiguousarray(X.astype(ml_dtypes.bfloat16))
    vertex = np.asarray(vertex).astype(np.int64)
    edges = np.asarray(edges).astype(np.int64)
    w_b = np.asarray(w_b, dtype=np.float32)
    w_a = np.asarray(w_a, dtype=np.float32)
    w_c = np.asarray(w_c, dtype=np.float32)
    b_b = np.asarray(b_b, dtype=np.float32).reshape(1, F)
    b_a = np.asarray(b_a, dtype=np.float32).reshape(1, F)
    b_c = np.asarray(b_c, dtype=np.float32).reshape(1, F)

    r = _route(vertex, edges)
    if r is None:
        return _numpy_fallback(X, vertex, edges, w_b, w_a, w_c, b_b, b_a, b_c)
    routed, cnt_a, cnt_c = r

    key = (cnt_a.tobytes(), cnt_c.tobytes())
    if _STATE.get("key") != key:
        _STATE["nc"] = _build_program(cnt_a, cnt_c)
        _STATE["key"] = key
    nc = _STATE["nc"]

    deg_full = np.bincount(vertex, minlength=N).astype(np.float32)
    wmats = {
        "wb1": w_b[:F], "wb2": w_b[F:],
        "wa1n": -w_a[:F], "wa2n": -w_a[F:],
        "wc1": w_c[:F], "wc2": w_c[F:],
    }
    bmats = {"bias_c": b_b, "bias_l": b_b - b_a, "bias_r": b_b + b_c}

    in_maps = []
    for m in range(NC):
        xs = np.zeros((NODE_SH_P, F), np.float32)
        xs[:NODE_SH] = X[m * NODE_SH:(m + 1) * NODE_SH]
        dshard = np.zeros(NODE_SH_P, np.float32)
        dshard[:NODE_SH] = deg_full[m * NODE_SH:(m + 1) * NODE_SH]
        im = {
            "xfull": Xb,
            "xshard": xs,
            "ia": routed[m]["ia"], "ic": routed[m]["ic"],
            "rowa": routed[m]["rowa"], "rowc": routed[m]["rowc"],
            "deg": np.ascontiguousarray(dshard.reshape(BLK_C, 128).T),
        }
        for nm, w in wmats.items():
            im[nm] = np.ascontiguousarray(w.astype(ml_dtypes.bfloat16))
        for nm, bv in bmats.items():
            im[nm] = np.ascontiguousarray(bv.astype(np.float32))
        in_maps.append(im)

    res = run_bass_kernel_spmd(nc, in_maps, list(range(NC)))
    full = np.concatenate([res.results[m]["out3"][:NODE_SH]
                           for m in range(NC)])
    full = full.reshape(N, 3, F)
    return (np.ascontiguousarray(full[:, 0]),
            np.ascontiguousarray(full[:, 1]),
            np.ascontiguousarray(full[:, 2]))


# revision 17
# speedup vs baseline: 1.2904x; 1.1643x over previous
"""Trainium2 Bass kernel for nn_CrispToFuzzyConv (hypergraph message passing).

v2: segment sums computed on the PE as one-hot matmuls (no DMA
scatter-adds, no DRAM accumulators, no zeroing):

  Stage A (edges sharded, 2 regions x 25 blocks of 128 edges/core):
    per (block, X-chunk): dma_gather 256 token slots of X[vertex]
    (tokens grouped by edge block; -1 padding costs no packets);
    onehot[p,j,c] = (c == local_edge_id[p,j]) built by one DVE
    tensor_tensor is_equal with broadcast APs;
    Xe_block = sum_j onehot_j^T @ dat_j accumulated in PSUM (f32r),
    stored to xe_sum. AllGather per region -> xe_tbl[r] [25600,128].
  Stage C (nodes sharded, 98 blocks of 128 nodes/core), two passes so
    the region-1 AllGather hides behind region-0 work:
    pass 1: gather 256 slots of Xe[edges] from xe_tbl[0], 2 matmuls
      dat_j^T @ onehot_j -> partial Xv2^T tile, parked in SBUF.
    pass 2: same for region 1 into PSUM, then the dense head:
      h2T = psum + partial (Xv2^T), h1T = transpose(deg * X tile),
      a*T = |.|, 6 f32r matmuls with [256,128] weights split in two,
      biases folded (bias_l = b_b - b_a with w_a negated), out3 write.

Known hardware constraints baked in:
  - gather indices are int16 -> X gathered in 4 chunks of 25000 rows;
    xe_tbl capped at 25600 rows; <= 1024 indices per call
  - gather layout: token t -> partition t%128, column-block t//128
  - trailing -1 indices are skipped (free padding); pad slots read
    stale SBUF, so dat pool buffers are memset once (0 * garbage
    would still be NaN if SBUF powers up with NaN bit patterns)
  - collective in/out tensors must be Internal, addr_space Local
"""

import os
import numpy as np

# ---------------------------------------------------------------- constants
N = 100000
E = 50000
NNZ = 300000
F = 128
NC = 8

EDGE_SH = E // NC            # 6250
NODE_SH = N // NC            # 12500
REG = EDGE_SH // 2           # 3125 edges per region
BLK_A = 25                   # 128-edge blocks per (core, region)
ROWS_REG = BLK_A * 128       # 3200 padded rows per (core, region)
XE_TBL = NC * ROWS_REG       # 25600 rows per region table (int16-safe)
CH = 4                       # X chunks (int16 gather limit)
CHROWS = N // CH             # 25000
KA = 256                     # slots per stage-A (block, chunk) gather
KC = 256                     # slots per stage-C (block, region) gather
BLK_C = 98                   # 128-node blocks per core
NODE_SH_P = BLK_C * 128      # 12544
SUB_A = 2 * CH               # 8 subtiles per A block
SUB_C = 4                    # 4 subtiles per C block
NG_A = 2 * BLK_A * CH        # 200 stage-A gather calls per core
NG_C = BLK_C * 2             # 196 stage-C gather calls per core

_STATE = {}


# ---------------------------------------------------------------- host side
def _wrap16(idx):
    """[n, K] int -> [n, 128, K//16] int16 (idx i at partition i%16, col
    i//16; replicated across the 8 groups of 16 partitions)."""
    n, K = idx.shape
    t = idx.reshape(n, K // 16, 16).transpose(0, 2, 1).astype(np.int16)
    return np.ascontiguousarray(np.tile(t, (1, 8, 1)))


def _route(vertex, edges):
    """Per-core gather idx + onehot rowid tensors, or None if any static
    capacity is exceeded (then the numpy fallback runs)."""
    le = edges % EDGE_SH
    owner_a = edges // EDGE_SH
    reg = le // REG
    loc_r = le - reg * REG
    blk_a = loc_r // 128
    row_a = (loc_r - blk_a * 128).astype(np.float32)
    chunk = vertex // CHROWS
    gidx_a = vertex - chunk * CHROWS
    owner_c = vertex // NODE_SH
    loc_c = vertex - owner_c * NODE_SH
    blk_c = loc_c // 128
    row_c = (loc_c - blk_c * 128).astype(np.float32)
    gidx_c = owner_a * ROWS_REG + loc_r

    out = []
    for m in range(NC):
        ia = np.full((NG_A, KA), -1, np.int64)
        na = np.zeros(NG_A, np.int64)
        rowa = np.full((NG_A // CH * SUB_A, 128), -1.0, np.float32)
        sel = np.nonzero(owner_a == m)[0]
        key = (reg[sel] * BLK_A + blk_a[sel]) * CH + chunk[sel]
        order = np.argsort(key, kind="stable")
        sel, ks = sel[order], key[order]
        starts = np.searchsorted(ks, np.arange(NG_A + 1))
        for g in range(NG_A):
            s = sel[starts[g]:starts[g + 1]]
            n = len(s)
            if n > KA:
                return None
            ia[g, :n] = gidx_a[s]
            na[g] = n
            rb, c = g // CH, g % CH
            slot = np.arange(n)
            rowa[rb * SUB_A + 2 * c + slot // 128, slot % 128] = row_a[s]
        ic = np.full((NG_C, KC), -1, np.int64)
        ncnt = np.zeros(NG_C, np.int64)
        rowc = np.full((BLK_C * SUB_C, 128), -1.0, np.float32)
        sel = np.nonzero(owner_c == m)[0]
        key = blk_c[sel] * 2 + reg[sel]
        order = np.argsort(key, kind="stable")
        sel, ks = sel[order], key[order]
        starts = np.searchsorted(ks, np.arange(NG_C + 1))
        for g in range(NG_C):
            s = sel[starts[g]:starts[g + 1]]
            n = len(s)
            if n > KC:
                return None
            ic[g, :n] = gidx_c[s]
            ncnt[g] = n
            b, r = g // 2, g % 2
            slot = np.arange(n)
            rowc[b * SUB_C + 2 * r + slot // 128, slot % 128] = row_c[s]
        out.append({
            "ia": ia, "na": na, "rowa": rowa,
            "ic": ic, "nc": ncnt, "rowc": rowc,
        })
    # equalize per-call counts across cores: num_idxs_reg is baked into
    # the (single, SPMD) program, so every core must issue the same
    # number of descriptors per call. Pad shorter cores with idx 0
    # (rowid stays -1 -> zero onehot column -> no contribution).
    cnt_a = np.maximum(np.max([o["na"] for o in out], axis=0), 16)
    cnt_c = np.maximum(np.max([o["nc"] for o in out], axis=0), 16)
    for o in out:
        for g in range(NG_A):
            o["ia"][g, o["na"][g]:cnt_a[g]] = 0
        for g in range(NG_C):
            o["ic"][g, o["nc"][g]:cnt_c[g]] = 0
        o["ia"] = _wrap16(o["ia"])
        o["ic"] = _wrap16(o["ic"])
        o["rowa"] = np.ascontiguousarray(o["rowa"].T)
        o["rowc"] = np.ascontiguousarray(o["rowc"].T)
    return out, cnt_a, cnt_c


def _numpy_fallback(X, vertex, edges, w_b, w_a, w_c, b_b, b_a, b_c):
    Xe = np.zeros((E, F), np.float32)
    np.add.at(Xe, edges, X[vertex])
    Xv2 = np.zeros((N, F), np.float32)
    np.add.at(Xv2, vertex, Xe[edges])
    deg = np.bincount(vertex, minlength=N).astype(np.float32)[:, None]
    Xv = np.concatenate([deg * X, Xv2], axis=1)
    center = Xv @ w_b + b_b
    aXv = np.abs(Xv)
    return (center.astype(np.float32),
            (center - (aXv @ w_a + b_a)).astype(np.float32),
            (center + (aXv @ w_c + b_c)).astype(np.float32))


# ------------------------------------------------------------- bass program
def _build_program(cnt_a, cnt_c):
    from concourse import bacc, tile
    import concourse.mybir as mybir

    f32 = mybir.dt.float32
    bf16 = mybir.dt.bfloat16
    i16 = mybir.dt.int16

    nc = bacc.Bacc(None, target_bir_lowering=False, debug=False,
                   num_devices=NC, num_swdge_queues=4)

    xfull = nc.dram_tensor("xfull", [N, F], bf16, kind="ExternalInput")
    xshard = nc.dram_tensor("xshard", [NODE_SH_P, F], f32, kind="ExternalInput")
    ia = nc.dram_tensor("ia", [NG_A, 128, KA // 16], i16, kind="ExternalInput")
    ic = nc.dram_tensor("ic", [NG_C, 128, KC // 16], i16, kind="ExternalInput")
    rowa_d = nc.dram_tensor("rowa", [128, NG_A // CH * SUB_A], f32,
                            kind="ExternalInput")
    rowc_d = nc.dram_tensor("rowc", [128, BLK_C * SUB_C], f32,
                            kind="ExternalInput")
    deg = nc.dram_tensor("deg", [128, BLK_C], f32, kind="ExternalInput")
    wts_d = {nm: nc.dram_tensor(nm, [F, F], bf16, kind="ExternalInput")
             for nm in ("wb1", "wb2", "wa1n", "wa2n", "wc1", "wc2")}
    bias_d = {nm: nc.dram_tensor(nm, [1, F], f32, kind="ExternalInput")
              for nm in ("bias_c", "bias_l", "bias_r")}
    out3 = nc.dram_tensor("out3", [NODE_SH_P, 3 * F], f32,
                          kind="ExternalOutput")

    xe_sum = nc.dram_tensor("xe_sum", [2 * ROWS_REG, F], bf16)
    xe_tbl = [nc.dram_tensor(f"xe_tbl{r}", [XE_TBL, F], bf16)
              for r in range(2)]

    eye_d = nc.inline_tensor(np.eye(128, dtype=np.float32), name="eye128")
    ramp_np = np.broadcast_to(
        np.arange(128, dtype=np.float32),
        (128, SUB_A, 128)).copy()
    ramp_d = nc.inline_tensor(ramp_np, name="ramp8")

    ISEQ = mybir.AluOpType.is_equal
    Abs = mybir.ActivationFunctionType.Abs
    Copy = mybir.ActivationFunctionType.Copy

    with tile.TileContext(nc) as tc:
        with (
            tc.tile_pool(name="cpool", bufs=1) as cpool,
            tc.tile_pool(name="ppool", bufs=1) as ppool,
            tc.tile_pool(name="ipool", bufs=16) as ipool,
            tc.tile_pool(name="dpa", bufs=6) as dpa,
            tc.tile_pool(name="oha", bufs=4) as oha,
            tc.tile_pool(name="dpc", bufs=6) as dpc,
            tc.tile_pool(name="ohc", bufs=4) as ohc,
            tc.tile_pool(name="spool", bufs=6) as spool,
            tc.tile_pool(name="opool", bufs=3) as opool,
            tc.tile_pool(name="ps_sg", bufs=3, space="PSUM") as ps_sg,
            tc.tile_pool(name="ps_tr", bufs=2, space="PSUM") as ps_tr,
            tc.tile_pool(name="ps_mm", bufs=3, space="PSUM") as ps_mm,
        ):
            # constants
            ident = cpool.tile([128, 128], f32)
            nc.sync.dma_start(ident[:], eye_d[:])
            ramp = cpool.tile([128, SUB_A, 128], f32)
            nc.sync.dma_start(ramp[:], ramp_d[:])
            rowa_s = cpool.tile([128, NG_A // CH * SUB_A], f32)
            nc.sync.dma_start(rowa_s[:], rowa_d[:])
            rowc_s = cpool.tile([128, BLK_C * SUB_C], f32)
            nc.sync.dma_start(rowc_s[:], rowc_d[:])
            deg_all = cpool.tile([128, BLK_C], f32)
            nc.sync.dma_start(deg_all[:], deg[:])
            ones = cpool.tile([1, F], f32)
            nc.vector.memset(ones[:], 1.0)
            wts = {}
            for nm, d in wts_d.items():
                wtile = cpool.tile([F, F], bf16, tag=nm)
                nc.sync.dma_start(wtile[:], d[:])
                wts[nm] = wtile
            bias_bc = {}
            bmm = ps_mm.tile([128, 3 * F], f32, tag="mm3")
            for k, (nm, d) in enumerate(bias_d.items()):
                btile = cpool.tile([1, F], f32, tag=nm)
                nc.sync.dma_start(btile[:], d[:])
                nc.tensor.matmul(bmm[:, k * F:(k + 1) * F], ones[:], btile[:],
                                 start=True, stop=True)
            for k, nm in enumerate(bias_d):
                bct = cpool.tile([128, F], f32, tag=f"bc_{nm}")
                nc.vector.tensor_copy(bct[:], bmm[:, k * F:(k + 1) * F])
                bias_bc[nm] = bct
            # pre-zero the gather data pools (pad slots are never written;
            # 0 * stale-NaN would poison PSUM)
            for _ in range(6):
                t = dpa.tile([128, SUB_A, F], bf16, tag="datA")
                nc.vector.memset(t[:], 0.0)
                t = dpc.tile([128, 2, F], bf16, tag="datC")
                nc.vector.memset(t[:], 0.0)

            def cc(r):
                lo, hi = r * ROWS_REG, (r + 1) * ROWS_REG
                nc.gpsimd.collective_compute(
                    "AllGather", mybir.AluOpType.bypass,
                    replica_groups=[list(range(NC))],
                    ins=[xe_sum[lo:hi, :].opt()],
                    outs=[xe_tbl[r].ap().opt()],
                )

            # stage A: Xe blocks via onehot matmuls
            for r in range(2):
                for b in range(BLK_A):
                    dat = dpa.tile([128, SUB_A, F], bf16, tag="datA")
                    for c in range(CH):
                        g = (r * BLK_A + b) * CH + c
                        it = ipool.tile([128, KA // 16], i16, tag="ita")
                        nc.sync.dma_start(it[:], ia[g])
                        nc.gpsimd.dma_gather(
                            dat[:, 2 * c:2 * c + 2, :],
                            xfull[c * CHROWS:(c + 1) * CHROWS, :],
                            it[:], KA, int(cnt_a[g]), F, queue_num=g % 4)
                    oh = oha.tile([128, SUB_A, 128], bf16, tag="ohA")
                    g0 = (r * BLK_A + b) * SUB_A
                    nc.vector.tensor_tensor(
                        oh[:], ramp[:],
                        rowa_s[:, g0:g0 + SUB_A].unsqueeze(2).broadcast_to(
                            (128, SUB_A, 128)),
                        ISEQ)
                    ps = ps_sg.tile([128, F], f32, tag="sg")
                    for j in range(SUB_A):
                        nc.tensor.matmul(ps[:], oh[:, j, :], dat[:, j, :],
                                         start=(j == 0), stop=(j == SUB_A - 1))
                    st = spool.tile([128, F], bf16, tag="xe_st")
                    nc.scalar.activation(st[:], ps[:], Copy)
                    row0 = r * ROWS_REG + b * 128
                    nc.scalar.dma_start(xe_sum[row0:row0 + 128, :], st[:])
                    if r == 1 and b == 1:
                        cc(0)
            cc(1)

            # stage C pass 1: region-0 partial Xv2^T into SBUF
            parts = []
            for b in range(BLK_C):
                dat = dpc.tile([128, 2, F], bf16, tag="datC")
                it = ipool.tile([128, KC // 16], i16, tag="itc")
                nc.sync.dma_start(it[:], ic[b * 2])
                nc.gpsimd.dma_gather(dat[:], xe_tbl[0][:], it[:],
                                     KC, int(cnt_c[b * 2]), F,
                                     queue_num=b % 4)
                oh = ohc.tile([128, 2, 128], bf16, tag="ohC")
                nc.vector.tensor_tensor(
                    oh[:], ramp[:, 0:2, :],
                    rowc_s[:, b * SUB_C:b * SUB_C + 2].unsqueeze(2)
                    .broadcast_to((128, 2, 128)),
                    ISEQ)
                ps = ps_sg.tile([128, 128], f32, tag="sg")
                for j in range(2):
                    nc.tensor.matmul(ps[:], dat[:, j, :], oh[:, j, :],
                                     start=(j == 0), stop=(j == 1))
                part = ppool.tile([128, 128], f32, tag=f"part{b}")
                nc.vector.tensor_copy(part[:], ps[:])
                parts.append(part)

            # stage C pass 2 + dense head
            for b in range(BLK_C):
                part = parts[b]
                dat = dpc.tile([128, 2, F], bf16, tag="datC")
                it = ipool.tile([128, KC // 16], i16, tag="itc")
                nc.sync.dma_start(it[:], ic[b * 2 + 1])
                nc.gpsimd.dma_gather(dat[:], xe_tbl[1][:], it[:],
                                     KC, int(cnt_c[b * 2 + 1]), F,
                                     queue_num=(b + 2) % 4)
                oh = ohc.tile([128, 2, 128], bf16, tag="ohC")
                nc.vector.tensor_tensor(
                    oh[:], ramp[:, 0:2, :],
                    rowc_s[:, b * SUB_C + 2:b * SUB_C + 4].unsqueeze(2)
                    .broadcast_to((128, 2, 128)),
                    ISEQ)
                ps = ps_sg.tile([128, 128], f32, tag="sg")
                for j in range(2):
                    nc.tensor.matmul(ps[:], dat[:, j, :], oh[:, j, :],
                                     start=(j == 0), stop=(j == 1))
                h2T = spool.tile([128, 128], bf16, tag="h2T")
                nc.vector.tensor_add(h2T[:], ps[:], part[:])
                a2T = spool.tile([128, 128], bf16, tag="a2T")
                nc.scalar.activation(a2T[:], h2T[:], Abs)

                r0 = b * 128
                xt = spool.tile([128, F], f32, tag="xt")
                nc.sync.dma_start(xt[:], xshard[r0:r0 + 128, :])
                h1 = spool.tile([128, F], f32, tag="h1")
                nc.vector.tensor_scalar_mul(h1[:], xt[:], deg_all[:, b:b + 1])
                h1T_ps = ps_tr.tile([128, F], f32, tag="h1T_ps")
                nc.tensor.transpose(h1T_ps[:], h1[:], ident[:])
                h1T = spool.tile([128, F], bf16, tag="h1T")
                nc.vector.tensor_copy(h1T[:], h1T_ps[:])
                a1T = spool.tile([128, F], bf16, tag="a1T")
                nc.scalar.activation(a1T[:], h1T_ps[:], Abs)

                groups = (
                    ("hsum_ps", (("h1T", "wb1"), ("h2T", "wb2"))),
                    ("lpart_ps", (("a1T", "wa1n"), ("a2T", "wa2n"))),
                    ("rpart_ps", (("a1T", "wc1"), ("a2T", "wc2"))),
                )
                lhs = {"h1T": h1T, "h2T": h2T, "a1T": a1T, "a2T": a2T}
                mm = ps_mm.tile([128, 3 * F], f32, tag="mm3")
                ps_out = {}
                for k, (psname, terms) in enumerate(groups):
                    pso = mm[:, k * F:(k + 1) * F]
                    for i, (ln, wn) in enumerate(terms):
                        nc.tensor.matmul(
                            pso, lhs[ln][:], wts[wn][:],
                            start=(i == 0), stop=(i == len(terms) - 1))
                    ps_out[psname] = pso
                ot = opool.tile([128, 3 * F], f32, tag="ot")
                nc.vector.tensor_add(ot[:, 0:F], bias_bc["bias_c"][:],
                                     ps_out["hsum_ps"])
                for k, (pname, bname) in enumerate((("lpart_ps", "bias_l"),
                                                    ("rpart_ps", "bias_r"))):
                    tmp = opool.tile([128, F], f32, tag=f"t{k}")
                    nc.vector.tensor_add(tmp[:], bias_bc[bname][:],
                                         ps_out[pname])
                    nc.vector.tensor_add(ot[:, (k + 1) * F:(k + 2) * F],
                                         tmp[:], ps_out["hsum_ps"])
                nc.sync.dma_start(out3[r0:r0 + 128, :], ot[:])

    nc.compile()
    return nc


# ------------------------------------------------------------------- driver
def kernel(X, vertex, edges, X0, n_edges, w_b, w_a, w_c, b_b, b_a, b_c):
    from concourse.bass_utils import run_bass_kernel_spmd

    import ml_dtypes
    X = np.ascontiguousarray(np.asarray(X, dtype=np.float32))
    Xb = np.ascontiguousarray(X.astype(ml_dtypes.bfloat16))
    vertex = np.asarray(vertex).astype(np.int64)
    edges = np.asarray(edges).astype(np.int64)
    w_b = np.asarray(w_b, dtype=np.float32)
    w_a = np.asarray(w_a, dtype=np.float32)
    w_c = np.asarray(w_c, dtype=np.float32)
    b_b = np.asarray(b_b, dtype=np.float32).reshape(1, F)
    b_a = np.asarray(b_a, dtype=np.float32).reshape(1, F)
    b_c = np.asarray(b_c, dtype=np.float32).reshape(1, F)

    r = _route(vertex, edges)
    if r is None:
        return _numpy_fallback(X, vertex, edges, w_b, w_a, w_c, b_b, b_a, b_c)
    routed, cnt_a, cnt_c = r

    key = (cnt_a.tobytes(), cnt_c.tobytes())
    if _STATE.get("key") != key:
        _STATE["nc"] = _build_program(cnt_a, cnt_c)
        _STATE["key"] = key
    nc = _STATE["nc"]

    deg_full = np.bincount(vertex, minlength=N).astype(np.float32)
    wmats = {
        "wb1": w_b[:F], "wb2": w_b[F:],
        "wa1n": -w_a[:F], "wa2n": -w_a[F:],
        "wc1": w_c[:F], "wc2": w_c[F:],
    }
    bmats = {"bias_c": b_b, "bias_l": b_b - b_a, "bias_r": b_b + b_c}

    in_maps = []
    for m in range(NC):
        xs = np.zeros((NODE_SH_P, F), np.float32)
        xs[:NODE_SH] = X[m * NODE_SH:(m + 1) * NODE_SH]
        dshard = np.zeros(NODE_SH_P, np.float32)
        dshard[:NODE_SH] = deg_full[m * NODE_SH:(m + 1) * NODE_SH]
        im = {
            "xfull": Xb,
            "xshard": xs,
            "ia": routed[m]["ia"], "ic": routed[m]["ic"],
            "rowa": routed[m]["rowa"], "rowc": routed[m]["rowc"],
            "deg": np.ascontiguousarray(dshard.reshape(BLK_C, 128).T),
        }
        for nm, w in wmats.items():
            im[nm] = np.ascontiguousarray(w.astype(ml_dtypes.bfloat16))
        for nm, bv in bmats.items():
            im[nm] = np.ascontiguousarray(bv.astype(np.float32))
        in_maps.append(im)

    res = run_bass_kernel_spmd(nc, in_maps, list(range(NC)))
    full = np.concatenate([res.results[m]["out3"][:NODE_SH]
                           for m in range(NC)])
    full = full.reshape(N, 3, F)
    return (np.ascontiguousarray(full[:, 0]),
            np.ascontiguousarray(full[:, 1]),
            np.ascontiguousarray(full[:, 2]))


# revision 18
# speedup vs baseline: 1.3056x; 1.0118x over previous
"""Trainium2 Bass kernel for nn_CrispToFuzzyConv (hypergraph message passing).

v2: segment sums computed on the PE as one-hot matmuls (no DMA
scatter-adds, no DRAM accumulators, no zeroing):

  Stage A (edges sharded, 2 regions x 25 blocks of 128 edges/core):
    per (block, X-chunk): dma_gather 256 token slots of X[vertex]
    (tokens grouped by edge block; -1 padding costs no packets);
    onehot[p,j,c] = (c == local_edge_id[p,j]) built by one DVE
    tensor_tensor is_equal with broadcast APs;
    Xe_block = sum_j onehot_j^T @ dat_j accumulated in PSUM (f32r),
    stored to xe_sum. AllGather per region -> xe_tbl[r] [25600,128].
  Stage C (nodes sharded, 98 blocks of 128 nodes/core), two passes so
    the region-1 AllGather hides behind region-0 work:
    pass 1: gather 256 slots of Xe[edges] from xe_tbl[0], 2 matmuls
      dat_j^T @ onehot_j -> partial Xv2^T tile, parked in SBUF.
    pass 2: same for region 1 into PSUM, then the dense head:
      h2T = psum + partial (Xv2^T), h1T = transpose(deg * X tile),
      a*T = |.|, 6 f32r matmuls with [256,128] weights split in two,
      biases folded (bias_l = b_b - b_a with w_a negated), out3 write.

Known hardware constraints baked in:
  - gather indices are int16 -> X gathered in 4 chunks of 25000 rows;
    xe_tbl capped at 25600 rows; <= 1024 indices per call
  - gather layout: token t -> partition t%128, column-block t//128
  - trailing -1 indices are skipped (free padding); pad slots read
    stale SBUF, so dat pool buffers are memset once (0 * garbage
    would still be NaN if SBUF powers up with NaN bit patterns)
  - collective in/out tensors must be Internal, addr_space Local
"""

import os
import numpy as np

# ---------------------------------------------------------------- constants
N = 100000
E = 50000
NNZ = 300000
F = 128
NC = 8

EDGE_SH = E // NC            # 6250
NODE_SH = N // NC            # 12500
REG = EDGE_SH // 2           # 3125 edges per region
BLK_A = 25                   # 128-edge blocks per (core, region)
ROWS_REG = BLK_A * 128       # 3200 padded rows per (core, region)
XE_TBL = NC * ROWS_REG       # 25600 rows per region table (int16-safe)
CH = 4                       # X chunks (int16 gather limit)
CHROWS = N // CH             # 25000
KA = 256                     # slots per stage-A (block, chunk) gather
KC = 256                     # slots per stage-C (block, region) gather
BLK_C = 98                   # 128-node blocks per core
NODE_SH_P = BLK_C * 128      # 12544
SUB_A = 2 * CH               # 8 subtiles per A block
SUB_C = 4                    # 4 subtiles per C block
NG_A = 2 * BLK_A * CH        # 200 stage-A gather calls per core
NG_C = BLK_C * 2             # 196 stage-C gather calls per core

_STATE = {}


# ---------------------------------------------------------------- host side
def _wrap16(idx):
    """[n, K] int -> [n, 128, K//16] int16 (idx i at partition i%16, col
    i//16; replicated across the 8 groups of 16 partitions)."""
    n, K = idx.shape
    t = idx.reshape(n, K // 16, 16).transpose(0, 2, 1).astype(np.int16)
    return np.ascontiguousarray(np.tile(t, (1, 8, 1)))


def _route(vertex, edges):
    """Per-core gather idx + onehot rowid tensors, or None if any static
    capacity is exceeded (then the numpy fallback runs)."""
    le = edges % EDGE_SH
    owner_a = edges // EDGE_SH
    reg = le // REG
    loc_r = le - reg * REG
    blk_a = loc_r // 128
    row_a = (loc_r - blk_a * 128).astype(np.float32)
    chunk = vertex // CHROWS
    gidx_a = vertex - chunk * CHROWS
    owner_c = vertex // NODE_SH
    loc_c = vertex - owner_c * NODE_SH
    blk_c = loc_c // 128
    row_c = (loc_c - blk_c * 128).astype(np.float32)
    gidx_c = owner_a * ROWS_REG + loc_r

    out = []
    for m in range(NC):
        ia = np.full((NG_A, KA), -1, np.int64)
        na = np.zeros(NG_A, np.int64)
        rowa = np.full((NG_A // CH * SUB_A, 128), -1.0, np.float32)
        sel = np.nonzero(owner_a == m)[0]
        key = (reg[sel] * BLK_A + blk_a[sel]) * CH + chunk[sel]
        order = np.argsort(key, kind="stable")
        sel, ks = sel[order], key[order]
        starts = np.searchsorted(ks, np.arange(NG_A + 1))
        for g in range(NG_A):
            s = sel[starts[g]:starts[g + 1]]
            n = len(s)
            if n > KA:
                return None
            ia[g, :n] = gidx_a[s]
            na[g] = n
            rb, c = g // CH, g % CH
            slot = np.arange(n)
            rowa[rb * SUB_A + 2 * c + slot // 128, slot % 128] = row_a[s]
        ic = np.full((NG_C, KC), -1, np.int64)
        ncnt = np.zeros(NG_C, np.int64)
        rowc = np.full((BLK_C * SUB_C, 128), -1.0, np.float32)
        sel = np.nonzero(owner_c == m)[0]
        key = blk_c[sel] * 2 + reg[sel]
        order = np.argsort(key, kind="stable")
        sel, ks = sel[order], key[order]
        starts = np.searchsorted(ks, np.arange(NG_C + 1))
        for g in range(NG_C):
            s = sel[starts[g]:starts[g + 1]]
            n = len(s)
            if n > KC:
                return None
            ic[g, :n] = gidx_c[s]
            ncnt[g] = n
            b, r = g // 2, g % 2
            slot = np.arange(n)
            rowc[b * SUB_C + 2 * r + slot // 128, slot % 128] = row_c[s]
        out.append({
            "ia": ia, "na": na, "rowa": rowa,
            "ic": ic, "nc": ncnt, "rowc": rowc,
        })
    # equalize per-call counts across cores: num_idxs_reg is baked into
    # the (single, SPMD) program, so every core must issue the same
    # number of descriptors per call. Pad shorter cores with idx 0
    # (rowid stays -1 -> zero onehot column -> no contribution).
    cnt_a = np.maximum(np.max([o["na"] for o in out], axis=0), 16)
    cnt_c = np.maximum(np.max([o["nc"] for o in out], axis=0), 16)
    for o in out:
        for g in range(NG_A):
            o["ia"][g, o["na"][g]:cnt_a[g]] = 0
        for g in range(NG_C):
            o["ic"][g, o["nc"][g]:cnt_c[g]] = 0
        o["ia"] = np.ascontiguousarray(
            _wrap16(o["ia"]).transpose(1, 0, 2).reshape(128, NG_A * (KA // 16)))
        o["ic"] = np.ascontiguousarray(
            _wrap16(o["ic"]).transpose(1, 0, 2).reshape(128, NG_C * (KC // 16)))
        o["rowa"] = np.ascontiguousarray(o["rowa"].T)
        o["rowc"] = np.ascontiguousarray(o["rowc"].T)
    return out, cnt_a, cnt_c


def _numpy_fallback(X, vertex, edges, w_b, w_a, w_c, b_b, b_a, b_c):
    Xe = np.zeros((E, F), np.float32)
    np.add.at(Xe, edges, X[vertex])
    Xv2 = np.zeros((N, F), np.float32)
    np.add.at(Xv2, vertex, Xe[edges])
    deg = np.bincount(vertex, minlength=N).astype(np.float32)[:, None]
    Xv = np.concatenate([deg * X, Xv2], axis=1)
    center = Xv @ w_b + b_b
    aXv = np.abs(Xv)
    return (center.astype(np.float32),
            (center - (aXv @ w_a + b_a)).astype(np.float32),
            (center + (aXv @ w_c + b_c)).astype(np.float32))


# ------------------------------------------------------------- bass program
def _build_program(cnt_a, cnt_c):
    from concourse import bacc, tile
    import concourse.mybir as mybir

    f32 = mybir.dt.float32
    bf16 = mybir.dt.bfloat16
    i16 = mybir.dt.int16

    nc = bacc.Bacc(None, target_bir_lowering=False, debug=False,
                   num_devices=NC, num_swdge_queues=4)

    xfull = nc.dram_tensor("xfull", [N, F], bf16, kind="ExternalInput")
    xshard = nc.dram_tensor("xshard", [NODE_SH_P, F], f32, kind="ExternalInput")
    ia = nc.dram_tensor("ia", [128, NG_A * (KA // 16)], i16, kind="ExternalInput")
    ic = nc.dram_tensor("ic", [128, NG_C * (KC // 16)], i16, kind="ExternalInput")
    rowa_d = nc.dram_tensor("rowa", [128, NG_A // CH * SUB_A], f32,
                            kind="ExternalInput")
    rowc_d = nc.dram_tensor("rowc", [128, BLK_C * SUB_C], f32,
                            kind="ExternalInput")
    deg = nc.dram_tensor("deg", [128, BLK_C], f32, kind="ExternalInput")
    wts_d = {nm: nc.dram_tensor(nm, [F, F], bf16, kind="ExternalInput")
             for nm in ("wb1", "wb2", "wa1n", "wa2n", "wc1", "wc2")}
    bias_d = {nm: nc.dram_tensor(nm, [1, F], f32, kind="ExternalInput")
              for nm in ("bias_c", "bias_l", "bias_r")}
    out3 = nc.dram_tensor("out3", [NODE_SH_P, 3 * F], f32,
                          kind="ExternalOutput")

    xe_sum = nc.dram_tensor("xe_sum", [2 * ROWS_REG, F], bf16)
    xe_tbl = [nc.dram_tensor(f"xe_tbl{r}", [XE_TBL, F], bf16)
              for r in range(2)]

    eye_d = nc.inline_tensor(np.eye(128, dtype=np.float32), name="eye128")
    ramp_np = np.broadcast_to(
        np.arange(128, dtype=np.float32),
        (128, SUB_A, 128)).copy()
    ramp_d = nc.inline_tensor(ramp_np, name="ramp8")

    ISEQ = mybir.AluOpType.is_equal
    Abs = mybir.ActivationFunctionType.Abs
    Copy = mybir.ActivationFunctionType.Copy

    with tile.TileContext(nc) as tc:
        with (
            tc.tile_pool(name="cpool", bufs=1) as cpool,
            tc.tile_pool(name="ppool", bufs=1) as ppool,
            tc.tile_pool(name="dpa", bufs=6) as dpa,
            tc.tile_pool(name="oha", bufs=4) as oha,
            tc.tile_pool(name="dpc", bufs=6) as dpc,
            tc.tile_pool(name="ohc", bufs=4) as ohc,
            tc.tile_pool(name="spool", bufs=6) as spool,
            tc.tile_pool(name="opool", bufs=3) as opool,
            tc.tile_pool(name="ps_sg", bufs=3, space="PSUM") as ps_sg,
            tc.tile_pool(name="ps_tr", bufs=2, space="PSUM") as ps_tr,
            tc.tile_pool(name="ps_mm", bufs=3, space="PSUM") as ps_mm,
        ):
            # constants
            ident = cpool.tile([128, 128], f32)
            nc.sync.dma_start(ident[:], eye_d[:])
            ramp = cpool.tile([128, SUB_A, 128], f32)
            nc.sync.dma_start(ramp[:], ramp_d[:])
            rowa_s = cpool.tile([128, NG_A // CH * SUB_A], f32)
            nc.sync.dma_start(rowa_s[:], rowa_d[:])
            rowc_s = cpool.tile([128, BLK_C * SUB_C], f32)
            nc.sync.dma_start(rowc_s[:], rowc_d[:])
            deg_all = cpool.tile([128, BLK_C], f32)
            nc.sync.dma_start(deg_all[:], deg[:])
            iat = cpool.tile([128, NG_A * (KA // 16)], i16)
            nc.sync.dma_start(iat[:], ia[:])
            ict = cpool.tile([128, NG_C * (KC // 16)], i16)
            nc.sync.dma_start(ict[:], ic[:])
            ones = cpool.tile([1, F], f32)
            nc.vector.memset(ones[:], 1.0)
            wts = {}
            for nm, d in wts_d.items():
                wtile = cpool.tile([F, F], bf16, tag=nm)
                nc.sync.dma_start(wtile[:], d[:])
                wts[nm] = wtile
            bias_bc = {}
            bmm = ps_mm.tile([128, 3 * F], f32, tag="mm3")
            for k, (nm, d) in enumerate(bias_d.items()):
                btile = cpool.tile([1, F], f32, tag=nm)
                nc.sync.dma_start(btile[:], d[:])
                nc.tensor.matmul(bmm[:, k * F:(k + 1) * F], ones[:], btile[:],
                                 start=True, stop=True)
            for k, nm in enumerate(bias_d):
                bct = cpool.tile([128, F], f32, tag=f"bc_{nm}")
                nc.vector.tensor_copy(bct[:], bmm[:, k * F:(k + 1) * F])
                bias_bc[nm] = bct
            # pre-zero the gather data pools (pad slots are never written;
            # 0 * stale-NaN would poison PSUM)
            for _ in range(6):
                t = dpa.tile([128, SUB_A, F], bf16, tag="datA")
                nc.vector.memset(t[:], 0.0)
                t = dpc.tile([128, 2, F], bf16, tag="datC")
                nc.vector.memset(t[:], 0.0)

            def cc(r):
                lo, hi = r * ROWS_REG, (r + 1) * ROWS_REG
                nc.gpsimd.collective_compute(
                    "AllGather", mybir.AluOpType.bypass,
                    replica_groups=[list(range(NC))],
                    ins=[xe_sum[lo:hi, :].opt()],
                    outs=[xe_tbl[r].ap().opt()],
                )

            # stage A: Xe blocks via onehot matmuls
            for r in range(2):
                for b in range(BLK_A):
                    dat = dpa.tile([128, SUB_A, F], bf16, tag="datA")
                    for c in range(CH):
                        g = (r * BLK_A + b) * CH + c
                        nc.gpsimd.dma_gather(
                            dat[:, 2 * c:2 * c + 2, :],
                            xfull[c * CHROWS:(c + 1) * CHROWS, :],
                            iat[:, g * (KA // 16):(g + 1) * (KA // 16)],
                            KA, int(cnt_a[g]), F, queue_num=g % 4)
                    oh = oha.tile([128, SUB_A, 128], bf16, tag="ohA")
                    g0 = (r * BLK_A + b) * SUB_A
                    nc.vector.tensor_tensor(
                        oh[:], ramp[:],
                        rowa_s[:, g0:g0 + SUB_A].unsqueeze(2).broadcast_to(
                            (128, SUB_A, 128)),
                        ISEQ)
                    ps = ps_sg.tile([128, F], f32, tag="sg")
                    for j in range(SUB_A):
                        nc.tensor.matmul(ps[:], oh[:, j, :], dat[:, j, :],
                                         start=(j == 0), stop=(j == SUB_A - 1))
                    st = spool.tile([128, F], bf16, tag="xe_st")
                    nc.scalar.activation(st[:], ps[:], Copy)
                    row0 = r * ROWS_REG + b * 128
                    nc.scalar.dma_start(xe_sum[row0:row0 + 128, :], st[:])
                    if r == 1 and b == 1:
                        cc(0)
            cc(1)

            # stage C pass 1: region-0 partial Xv2^T into SBUF
            parts = []
            for b in range(BLK_C):
                dat = dpc.tile([128, 2, F], bf16, tag="datC")
                g = b * 2
                nc.gpsimd.dma_gather(dat[:], xe_tbl[0][:],
                                     ict[:, g * (KC // 16):(g + 1) * (KC // 16)],
                                     KC, int(cnt_c[g]), F,
                                     queue_num=b % 4)
                oh = ohc.tile([128, 2, 128], bf16, tag="ohC")
                nc.vector.tensor_tensor(
                    oh[:], ramp[:, 0:2, :],
                    rowc_s[:, b * SUB_C:b * SUB_C + 2].unsqueeze(2)
                    .broadcast_to((128, 2, 128)),
                    ISEQ)
                ps = ps_sg.tile([128, 128], f32, tag="sg")
                for j in range(2):
                    nc.tensor.matmul(ps[:], dat[:, j, :], oh[:, j, :],
                                     start=(j == 0), stop=(j == 1))
                part = ppool.tile([128, 128], f32, tag=f"part{b}")
                nc.vector.tensor_copy(part[:], ps[:])
                parts.append(part)

            # stage C pass 2 + dense head
            for b in range(BLK_C):
                part = parts[b]
                dat = dpc.tile([128, 2, F], bf16, tag="datC")
                g = b * 2 + 1
                nc.gpsimd.dma_gather(dat[:], xe_tbl[1][:],
                                     ict[:, g * (KC // 16):(g + 1) * (KC // 16)],
                                     KC, int(cnt_c[g]), F,
                                     queue_num=(b + 2) % 4)
                oh = ohc.tile([128, 2, 128], bf16, tag="ohC")
                nc.vector.tensor_tensor(
                    oh[:], ramp[:, 0:2, :],
                    rowc_s[:, b * SUB_C + 2:b * SUB_C + 4].unsqueeze(2)
                    .broadcast_to((128, 2, 128)),
                    ISEQ)
                ps = ps_sg.tile([128, 128], f32, tag="sg")
                for j in range(2):
                    nc.tensor.matmul(ps[:], dat[:, j, :], oh[:, j, :],
                                     start=(j == 0), stop=(j == 1))
                h2T = spool.tile([128, 128], bf16, tag="h2T")
                nc.vector.tensor_add(h2T[:], ps[:], part[:])
                a2T = spool.tile([128, 128], bf16, tag="a2T")
                nc.scalar.activation(a2T[:], h2T[:], Abs)

                r0 = b * 128
                xt = spool.tile([128, F], f32, tag="xt")
                nc.sync.dma_start(xt[:], xshard[r0:r0 + 128, :])
                h1 = spool.tile([128, F], f32, tag="h1")
                nc.vector.tensor_scalar_mul(h1[:], xt[:], deg_all[:, b:b + 1])
                h1T_ps = ps_tr.tile([128, F], f32, tag="h1T_ps")
                nc.tensor.transpose(h1T_ps[:], h1[:], ident[:])
                h1T = spool.tile([128, F], bf16, tag="h1T")
                nc.vector.tensor_copy(h1T[:], h1T_ps[:])
                a1T = spool.tile([128, F], bf16, tag="a1T")
                nc.scalar.activation(a1T[:], h1T_ps[:], Abs)

                groups = (
                    ("hsum_ps", (("h1T", "wb1"), ("h2T", "wb2"))),
                    ("lpart_ps", (("a1T", "wa1n"), ("a2T", "wa2n"))),
                    ("rpart_ps", (("a1T", "wc1"), ("a2T", "wc2"))),
                )
                lhs = {"h1T": h1T, "h2T": h2T, "a1T": a1T, "a2T": a2T}
                mm = ps_mm.tile([128, 3 * F], f32, tag="mm3")
                ps_out = {}
                for k, (psname, terms) in enumerate(groups):
                    pso = mm[:, k * F:(k + 1) * F]
                    for i, (ln, wn) in enumerate(terms):
                        nc.tensor.matmul(
                            pso, lhs[ln][:], wts[wn][:],
                            start=(i == 0), stop=(i == len(terms) - 1))
                    ps_out[psname] = pso
                ot = opool.tile([128, 3 * F], f32, tag="ot")
                nc.vector.tensor_add(ot[:, 0:F], bias_bc["bias_c"][:],
                                     ps_out["hsum_ps"])
                for k, (pname, bname) in enumerate((("lpart_ps", "bias_l"),
                                                    ("rpart_ps", "bias_r"))):
                    tmp = opool.tile([128, F], f32, tag=f"t{k}")
                    nc.vector.tensor_add(tmp[:], bias_bc[bname][:],
                                         ps_out[pname])
                    nc.vector.tensor_add(ot[:, (k + 1) * F:(k + 2) * F],
                                         tmp[:], ps_out["hsum_ps"])
                nc.sync.dma_start(out3[r0:r0 + 128, :], ot[:])

    nc.compile()
    return nc


# ------------------------------------------------------------------- driver
def kernel(X, vertex, edges, X0, n_edges, w_b, w_a, w_c, b_b, b_a, b_c):
    from concourse.bass_utils import run_bass_kernel_spmd

    import ml_dtypes
    X = np.ascontiguousarray(np.asarray(X, dtype=np.float32))
    Xb = np.ascontiguousarray(X.astype(ml_dtypes.bfloat16))
    vertex = np.asarray(vertex).astype(np.int64)
    edges = np.asarray(edges).astype(np.int64)
    w_b = np.asarray(w_b, dtype=np.float32)
    w_a = np.asarray(w_a, dtype=np.float32)
    w_c = np.asarray(w_c, dtype=np.float32)
    b_b = np.asarray(b_b, dtype=np.float32).reshape(1, F)
    b_a = np.asarray(b_a, dtype=np.float32).reshape(1, F)
    b_c = np.asarray(b_c, dtype=np.float32).reshape(1, F)

    r = _route(vertex, edges)
    if r is None:
        return _numpy_fallback(X, vertex, edges, w_b, w_a, w_c, b_b, b_a, b_c)
    routed, cnt_a, cnt_c = r

    key = (cnt_a.tobytes(), cnt_c.tobytes())
    if _STATE.get("key") != key:
        _STATE["nc"] = _build_program(cnt_a, cnt_c)
        _STATE["key"] = key
    nc = _STATE["nc"]

    deg_full = np.bincount(vertex, minlength=N).astype(np.float32)
    wmats = {
        "wb1": w_b[:F], "wb2": w_b[F:],
        "wa1n": -w_a[:F], "wa2n": -w_a[F:],
        "wc1": w_c[:F], "wc2": w_c[F:],
    }
    bmats = {"bias_c": b_b, "bias_l": b_b - b_a, "bias_r": b_b + b_c}

    in_maps = []
    for m in range(NC):
        xs = np.zeros((NODE_SH_P, F), np.float32)
        xs[:NODE_SH] = X[m * NODE_SH:(m + 1) * NODE_SH]
        dshard = np.zeros(NODE_SH_P, np.float32)
        dshard[:NODE_SH] = deg_full[m * NODE_SH:(m + 1) * NODE_SH]
        im = {
            "xfull": Xb,
            "xshard": xs,
            "ia": routed[m]["ia"], "ic": routed[m]["ic"],
            "rowa": routed[m]["rowa"], "rowc": routed[m]["rowc"],
            "deg": np.ascontiguousarray(dshard.reshape(BLK_C, 128).T),
        }
        for nm, w in wmats.items():
            im[nm] = np.ascontiguousarray(w.astype(ml_dtypes.bfloat16))
        for nm, bv in bmats.items():
            im[nm] = np.ascontiguousarray(bv.astype(np.float32))
        in_maps.append(im)

    res = run_bass_kernel_spmd(nc, in_maps, list(range(NC)))
    full = np.concatenate([res.results[m]["out3"][:NODE_SH]
                           for m in range(NC)])
    full = full.reshape(N, 3, F)
    return (np.ascontiguousarray(full[:, 0]),
            np.ascontiguousarray(full[:, 1]),
            np.ascontiguousarray(full[:, 2]))
